# revision 18
# baseline (speedup 1.0000x reference)
"""Trainium2 Bass kernel for nn_AttLayer (sparse windowed attention).

Contract: kernel(**inputs) takes the FULL unsharded inputs (numpy, fp32)
and returns the FULL output (1, 512, 32768) fp32. Internally shards the
sequence (block axis) across 8 NeuronCores: each core owns 64 blocks of
64 frames (4096 frames) plus a 32-frame K/V halo on each side.

Math per core (Cp = 256, bl = 64):
  qc = Wq q + bq ; kc = Wk k + bk           (channel-major, bf16)
  vt = (Wv v + bv)^T                        (frame-major, bf16)
  per block j:  e = qc_j^T kc_win / 16      (64 x 128, fp32 psum)
                n = exp(e) * (mask + 1e-6); D = rowsum(n)  (fused DVE)
                m = n / D  (bf16) ; attT = transpose(m)    (PE)
                av = vt_win^T attT                          (psum)
  rav = relu(av) (bf16);  out = Wo rav + bo  (fp32)

Self-contained: hardcodes all shapes; imports only concourse + numpy.
"""

import sys
from contextlib import ExitStack

import numpy as np

for _p in ("/opt/trn_rl_repo", "/root/.axon_site/_ro/trn_rl_repo"):
    if _p not in sys.path:
        sys.path.append(_p)

import ml_dtypes  # noqa: E402

import concourse.bass as bass  # noqa: E402
from concourse import bacc  # noqa: E402
import concourse.mybir as mybir  # noqa: E402
import concourse.tile as tile  # noqa: E402

BF16 = ml_dtypes.bfloat16
F32 = mybir.dt.float32
BF = mybir.dt.bfloat16
Alu = mybir.AluOpType
Act = mybir.ActivationFunctionType

C = 512          # channels
CP = 256         # reduced channels
L = 32768        # full sequence
NCORES = 8
LLOC = L // NCORES          # 4096 frames per core
LKV = LLOC + 64             # 4160 k/v frames per core (32 halo each side)
BL = 64                     # block size
NCHUNK = 8                  # chunks per core
CH = 512                    # frames per chunk
NPAIR_C = 4                 # block-pairs per chunk

_NC_CACHE = {}


def _build_nc(reps=1, loop_k=None):
    nc = bacc.Bacc()
    d = {}
    inp = lambda name, shape, dt: nc.declare_dram_parameter(name, list(shape), dt, isOutput=False)
    d["qs"] = inp("qs", (C, LLOC), BF)
    d["ks"] = inp("ks", (C, LKV), BF)
    d["vs"] = inp("vs", (C, LKV), BF)
    d["wqt"] = inp("wqt", (C, CP), BF)
    d["wkt"] = inp("wkt", (C, CP), BF)
    d["wvt"] = inp("wvt", (C, CP), BF)
    d["wot"] = inp("wot", (CP, C), BF)
    d["bq2"] = inp("bq2", (CP, 1), F32)
    d["bk2"] = inp("bk2", (CP, 1), F32)
    d["bo4"] = inp("bo4", (C, 1), F32)
    d["bvb"] = inp("bvb", (128, CP), BF)
    d["w0"] = inp("w0", (128, 128), BF)
    d["wedge"] = inp("wedge", (2, 128, 128), BF)
    d["ident"] = inp("ident", (128, 128), BF)
    d["os"] = nc.declare_dram_parameter("os", [C, LLOC], F32, isOutput=True)

    with tile.TileContext(nc) as tc:
        with ExitStack() as ctx:
            _emit(ctx, tc, nc, d, reps, loop_k)
    nc.finalize()
    return nc


def _emit(ctx, tc, nc, d, reps, loop_k=None):
    # DRAM tiled views
    qs_v = d["qs"][:].rearrange("(t p) l -> p t l", p=128)   # [128, 4, 4096]
    ks_v = d["ks"][:].rearrange("(t p) l -> p t l", p=128)   # [128, 4, 4160]
    vs_v = d["vs"][:].rearrange("(t p) l -> p t l", p=128)
    os_v = d["os"][:].rearrange("(m p) l -> p m l", p=128)   # [128, 4, 4096]

    const = ctx.enter_context(tc.tile_pool(name="const", bufs=1))
    p_qin = ctx.enter_context(tc.tile_pool(name="qin", bufs=3))
    p_kin = ctx.enter_context(tc.tile_pool(name="kin", bufs=3))
    p_vin = ctx.enter_context(tc.tile_pool(name="vin", bufs=3))
    p_qc = ctx.enter_context(tc.tile_pool(name="qc", bufs=3))
    p_kc = ctx.enter_context(tc.tile_pool(name="kc", bufs=3))
    p_vt = ctx.enter_context(tc.tile_pool(name="vt", bufs=10))
    p_vts = ctx.enter_context(tc.tile_pool(name="vts", bufs=6))
    p_exp = ctx.enter_context(tc.tile_pool(name="exp", bufs=4))
    p_n = ctx.enter_context(tc.tile_pool(name="nn", bufs=4))
    p_dr = ctx.enter_context(tc.tile_pool(name="dr", bufs=6))
    p_m = ctx.enter_context(tc.tile_pool(name="mm", bufs=4))
    p_att = ctx.enter_context(tc.tile_pool(name="attT", bufs=4))
    p_rav = ctx.enter_context(tc.tile_pool(name="rav", bufs=3))
    p_out = ctx.enter_context(tc.tile_pool(name="outs", bufs=3))

    ps_qk = ctx.enter_context(tc.tile_pool(name="ps_qk", bufs=2, space="PSUM"))
    ps_v = ctx.enter_context(tc.tile_pool(name="ps_v", bufs=1, space="PSUM"))
    ps_en = ctx.enter_context(tc.tile_pool(name="ps_en", bufs=2, space="PSUM"))
    ps_mt = ctx.enter_context(tc.tile_pool(name="ps_mt", bufs=1, space="PSUM"))
    ps_av = ctx.enter_context(tc.tile_pool(name="ps_av", bufs=1, space="PSUM"))
    ps_fin = ctx.enter_context(tc.tile_pool(name="ps_fin", bufs=1, space="PSUM"))

    # ---- constants ----
    wqt_sb = const.tile([128, 4, CP], BF, tag="wqt")
    nc.sync.dma_start(out=wqt_sb[:], in_=d["wqt"][:].rearrange("(t p) c -> p t c", p=128))
    wkt_sb = const.tile([128, 4, CP], BF, tag="wkt")
    nc.sync.dma_start(out=wkt_sb[:], in_=d["wkt"][:].rearrange("(t p) c -> p t c", p=128))
    wvt_sb = const.tile([128, 4, CP], BF, tag="wvt")
    nc.sync.dma_start(out=wvt_sb[:], in_=d["wvt"][:].rearrange("(t p) c -> p t c", p=128))
    wot_sb = const.tile([128, 2, C], BF, tag="wot")
    nc.sync.dma_start(out=wot_sb[:], in_=d["wot"][:].rearrange("(t p) c -> p t c", p=128))
    bq_sb = const.tile([128, 2], F32, tag="bq")
    nc.sync.dma_start(out=bq_sb[:], in_=d["bq2"][:].rearrange("(t p) o -> p (t o)", p=128))
    bk_sb = const.tile([128, 2], F32, tag="bk")
    nc.sync.dma_start(out=bk_sb[:], in_=d["bk2"][:].rearrange("(t p) o -> p (t o)", p=128))
    bo_sb = const.tile([128, 4], F32, tag="bo")
    nc.sync.dma_start(out=bo_sb[:], in_=d["bo4"][:].rearrange("(t p) o -> p (t o)", p=128))
    bvr_sb = const.tile([1, CP], BF, tag="bvr")
    nc.sync.dma_start(out=bvr_sb[:], in_=d["bvr"][:])
    w0_sb = const.tile([128, 128], F32, tag="w0")
    nc.sync.dma_start(out=w0_sb[:], in_=d["w0"][:])
    wedge_sb = const.tile([128, 2, 128], F32, tag="wedge")
    nc.sync.dma_start(out=wedge_sb[:], in_=d["wedge"][:].rearrange("e p k -> p e k"))
    ident_sb = const.tile([128, 128], BF, tag="ident")
    nc.sync.dma_start(out=ident_sb[:], in_=d["ident"][:])
    ones_sb = const.tile([1, 128], BF, tag="ones")
    nc.vector.memset(ones_sb[:], 1.0)

    args = (tc, nc, d, qs_v, ks_v, vs_v, os_v,
            wqt_sb, wkt_sb, wvt_sb, wot_sb, bq_sb, bk_sb, bo_sb,
            bvb_sb, w0_sb, wedge_sb, ident_sb,
            p_qin, p_kin, p_vin, p_qc, p_kc, p_vt, p_vts,
            p_exp, p_n, p_dr, p_m, p_att, p_rav, p_out,
            ps_qk, ps_v, ps_en, ps_mt, ps_av, ps_fin)
    if loop_k is not None and loop_k > 1:
        with tc.For_i(0, loop_k, 1):
            _emit_body(*args)
    else:
        for _rep in range(reps):
            _emit_body(*args)


def _emit_body(tc, nc, d, qs_v, ks_v, vs_v, os_v,
               wqt_sb, wkt_sb, wvt_sb, wot_sb, bq_sb, bk_sb, bo_sb,
               bvb_sb, w0_sb, wedge_sb, ident_sb,
               p_qin, p_kin, p_vin, p_qc, p_kc, p_vt, p_vts,
               p_exp, p_n, p_dr, p_m, p_att, p_rav, p_out,
               ps_qk, ps_v, ps_en, ps_mt, ps_av, ps_fin):
    kc_tiles = {}     # chunk -> sbuf [128, 2, 576] bf16
    vt_tiles = {}     # 128-frame tile index -> sbuf [128, 256] bf16

    def load_kv(cc):
        """DMA k/v input chunk cc (512 frames, or 64-frame tail for cc==8)."""
        w = CH if cc < NCHUNK else 64
        kin = p_kin.tile([128, 4, CH], BF, tag="kin")
        nc.sync.dma_start(out=kin[:, :, :w], in_=ks_v[:, :, cc * CH:cc * CH + w])
        vin = p_vin.tile([128, 4, CH], BF, tag="vin")
        nc.sync.dma_start(out=vin[:, :, :w], in_=vs_v[:, :, cc * CH:cc * CH + w])
        return kin, vin

    def conv_k(cc, kin):
        """K-conv for chunk cc; fills kc_tiles[cc][:, :, 0:512] and patches
        kc_tiles[cc-1][:, :, 512:576]."""
        w = CH if cc < NCHUNK else 64
        if cc < NCHUNK:
            kc_tiles[cc] = p_kc.tile([128, 2, CH + 64], BF, tag="kc")
        for cpt in range(2):
            ps = ps_qk.tile([128, CH], F32, tag="psqk")
            for ct in range(4):
                nc.tensor.matmul(ps[:, :w],
                                 wkt_sb[:, ct, cpt * 128:(cpt + 1) * 128],
                                 kin[:, ct, :w],
                                 start=(ct == 0), stop=(ct == 3))
            if cc < NCHUNK:
                nc.scalar.activation(kc_tiles[cc][:, cpt, 0:CH], ps[:],
                                     Act.Identity, bias=bk_sb[:, cpt:cpt + 1])
                if cc > 0:
                    # patch previous chunk's 64-col tail
                    nc.vector.tensor_copy(kc_tiles[cc - 1][:, cpt, CH:CH + 64],
                                          kc_tiles[cc][:, cpt, 0:64])
            else:
                # 64-frame tail chunk: write directly into chunk 7's tail
                nc.scalar.activation(kc_tiles[cc - 1][:, cpt, CH:CH + 64], ps[:, 0:64],
                                     Act.Identity, bias=bk_sb[:, cpt:cpt + 1])

    def conv_v(cc, vin):
        """V-conv (transposed output) for chunk cc; fills vt_tiles[4*cc + t]."""
        nlt = 4 if cc < NCHUNK else 1
        w = 128 if cc < NCHUNK else 64
        for t in range(nlt):
            ps = ps_v.tile([128, CP], F32, tag="psv")
            for ct in range(4):
                nc.tensor.matmul(ps[:w, :],
                                 vin[:, ct, t * 128:t * 128 + w],
                                 wvt_sb[:, ct, :],
                                 start=(ct == 0), stop=False)
            nc.tensor.matmul(ps[:w, :], ones_sb[:, :w], bvr_sb[:],
                             start=False, stop=True)
            vt = p_vt.tile([128, CP], BF, tag="vt")
            nc.vector.tensor_copy(vt[:w, :], ps[:w, :])
            vt_tiles[4 * cc + t] = vt

    # ---- prologue: chunk 0 k/v ----
    kin, vin = load_kv(0)
    conv_k(0, kin)
    conv_v(0, vin)

    for c in range(NCHUNK):
        # 1) k/v conv one chunk ahead (incl. 64-frame tail at c == 7)
        kin, vin = load_kv(c + 1)
        conv_k(c + 1, kin)
        conv_v(c + 1, vin)

        # 2) shifted v tiles for the odd blocks of chunk c
        vts_tiles = {}
        for p in range(NPAIR_C):
            P = 4 * c + p
            vts = p_vts.tile([128, CP], BF, tag="vts")
            nc.gpsimd.dma_start(out=vts[0:64, :], in_=vt_tiles[P][64:128, :])
            nc.gpsimd.dma_start(out=vts[64:128, :], in_=vt_tiles[P + 1][0:64, :])
            vts_tiles[P] = vts

        # 3) q conv for chunk c
        qin = p_qin.tile([128, 4, CH], BF, tag="qin")
        nc.gpsimd.dma_start(out=qin[:], in_=qs_v[:, :, c * CH:(c + 1) * CH])
        qc = p_qc.tile([128, 2, CH], BF, tag="qc")
        for cpt in range(2):
            ps = ps_qk.tile([128, CH], F32, tag="psqk")
            for ct in range(4):
                nc.tensor.matmul(ps[:],
                                 wqt_sb[:, ct, cpt * 128:(cpt + 1) * 128],
                                 qin[:, ct, :],
                                 start=(ct == 0), stop=(ct == 3))
            nc.scalar.activation(qc[:, cpt, :], ps[:], Act.Identity,
                                 bias=bq_sb[:, cpt:cpt + 1])

        # 4) attention per block-pair
        kc = kc_tiles[c]
        rav = p_rav.tile([128, 2, CH], BF, tag="rav")
        for p in range(NPAIR_C):
            P = 4 * c + p
            # energy for the two blocks, stacked on partitions
            ps_e = ps_en.tile([128, 128], F32, tag="psen")
            for half in range(2):
                jj = 2 * p + half
                for cpt in range(2):
                    nc.tensor.matmul(ps_e[half * 64:(half + 1) * 64, :],
                                     qc[:, cpt, jj * 64:jj * 64 + 64],
                                     kc[:, cpt, jj * 64:jj * 64 + 128],
                                     start=(cpt == 0), stop=(cpt == 1))
            # softmax (unnormalized weights n, row-sums D via fused op)
            if P == 0:
                Wsel = wedge_sb[:, 0, :]
            elif P == NCHUNK * NPAIR_C - 1:
                Wsel = wedge_sb[:, 1, :]
            else:
                Wsel = w0_sb[:]
            expt = p_exp.tile([128, 128], F32, tag="exp")
            nc.scalar.activation(expt[:], ps_e[:], Act.Exp, scale=1.0 / 16.0)
            nt = p_n.tile([128, 128], F32, tag="nn")
            Dt = p_dr.tile([128, 1], F32, tag="D")
            nc.vector.scalar_tensor_tensor(nt[:], expt[:], 0.0, Wsel,
                                           op0=Alu.bypass, op1=Alu.mult,
                                           accum_out=Dt[:])
            rt = p_dr.tile([128, 1], F32, tag="r")
            nc.vector.reciprocal(rt[:], Dt[:])
            mt = p_m.tile([128, 128], BF, tag="mm")
            nc.vector.tensor_scalar_mul(mt[:], nt[:], rt[:])
            # transpose att (PE) then copy to SBUF
            ps_m = ps_mt.tile([128, 128], BF, tag="psmt")
            nc.tensor.transpose(ps_m[:], mt[:], ident_sb[:])
            attT = p_att.tile([128, 128], BF, tag="attT")
            nc.vector.tensor_copy(attT[:], ps_m[:])
            # att @ V  -> [cp, 128] for the pair
            ps_a = ps_av.tile([128, 2, 128], F32, tag="psav")
            for cpt in range(2):
                nc.tensor.matmul(ps_a[:, cpt, 0:64],
                                 vt_tiles[P][:, cpt * 128:(cpt + 1) * 128],
                                 attT[:, 0:64], start=True, stop=True)
                nc.tensor.matmul(ps_a[:, cpt, 64:128],
                                 vts_tiles[P][:, cpt * 128:(cpt + 1) * 128],
                                 attT[:, 64:128], start=True, stop=True)
            # relu -> rav (alternate engines for balance)
            for cpt in range(2):
                dst = rav[:, cpt, p * 128:(p + 1) * 128]
                nc.vector.tensor_scalar_max(dst, ps_a[:, cpt, :], 0.0)

        # 5) output conv
        outs = p_out.tile([128, 4, CH], F32, tag="outs")
        for m in range(4):
            ps = ps_fin.tile([128, CH], F32, tag="psfin")
            for kt in range(2):
                nc.tensor.matmul(ps[:],
                                 wot_sb[:, kt, m * 128:(m + 1) * 128],
                                 rav[:, kt, :],
                                 start=(kt == 0), stop=(kt == 1))
            if m < 2:
                nc.scalar.activation(outs[:, m, :], ps[:], Act.Identity,
                                     bias=bo_sb[:, m:m + 1])
            else:
                nc.vector.tensor_scalar_add(outs[:, m, :], ps[:], bo_sb[:, m:m + 1])
        nc.sync.dma_start(out=os_v[:, 0:2, c * CH:(c + 1) * CH], in_=outs[:, 0:2, :])
        nc.scalar.dma_start(out=os_v[:, 2:4, c * CH:(c + 1) * CH], in_=outs[:, 2:4, :])


def _window_mask():
    i = np.arange(BL)[:, None]
    j = np.arange(2 * BL)[None, :]
    return ((j >= i) & (j < i + BL)).astype(np.float32)


def _host_shard(q, k, v, mask, Wq, bq, Wk, bk, Wv, bv, Wo, bo):
    half = BL // 2
    q0 = np.asarray(q[0], np.float32)
    kp = np.pad(np.asarray(k[0], np.float32), ((0, 0), (half, half)))
    vp = np.pad(np.asarray(v[0], np.float32), ((0, 0), (half, half)))
    pmp = np.pad(np.asarray(mask[0, 0], np.float32), (half, half))
    wm = _window_mask()
    w0 = (np.concatenate([wm, wm], axis=0) + 1e-6).astype(BF16)
    shared = {
        "wqt": np.ascontiguousarray(Wq.T).astype(BF16),
        "wkt": np.ascontiguousarray(Wk.T).astype(BF16),
        "wvt": np.ascontiguousarray(Wv.T).astype(BF16),
        "wot": np.ascontiguousarray(Wo.T).astype(BF16),
        "bq2": np.asarray(bq, np.float32).reshape(CP, 1),
        "bk2": np.asarray(bk, np.float32).reshape(CP, 1),
        "bo4": np.asarray(bo, np.float32).reshape(C, 1),
        "bvb": np.tile(np.asarray(bv, np.float32).reshape(1, CP), (128, 1)).astype(BF16),
        "w0": w0,
        "ident": np.eye(128, dtype=BF16),
    }
    in_maps = []
    for r in range(NCORES):
        s = r * LLOC
        pml = pmp[s:s + LKV]

        def wpair(P):
            r0 = wm * pml[128 * P:128 * P + 128][None, :]
            r1 = wm * pml[128 * P + 64:128 * P + 192][None, :]
            return np.concatenate([r0, r1], axis=0) + 1e-6

        wedge = np.stack([wpair(0), wpair(31)]).astype(BF16)
        in_maps.append({
            "qs": q0[:, s:s + LLOC].astype(BF16),
            "ks": kp[:, s:s + LKV].astype(BF16),
            "vs": vp[:, s:s + LKV].astype(BF16),
            "wedge": wedge,
            **shared,
        })
    return in_maps


def kernel(q, k, v, mask, Wq, bq, Wk, bk, Wv, bv, Wo, bo):
    from concourse.bass_utils import run_bass_kernel_spmd

    if "nc" not in _NC_CACHE:
        _NC_CACHE["nc"] = _build_nc()
    nc = _NC_CACHE["nc"]
    in_maps = _host_shard(q, k, v, mask, Wq, bq, Wk, bk, Wv, bv, Wo, bo)
    res = run_bass_kernel_spmd(nc, in_maps, list(range(NCORES)))
    out = np.concatenate([res.results[r]["os"] for r in range(NCORES)], axis=1)
    out = out[None].astype(np.float32)
    return out * np.asarray(mask, np.float32)[:, 0:1, :]


# revision 21
# speedup vs baseline: 1.0901x; 1.0901x over previous
"""Trainium2 Bass kernel for nn_AttLayer (sparse windowed attention).

Contract: kernel(**inputs) takes the FULL unsharded inputs (numpy, fp32)
and returns the FULL output (1, 512, 32768) fp32. Internally shards the
sequence (block axis) across 8 NeuronCores: each core owns 64 blocks of
64 frames (4096 frames) plus a 32-frame K/V halo on each side.

Math per core (Cp = 256, bl = 64):
  qc = Wq q + bq ; kc = Wk k + bk           (channel-major, bf16)
  vt = (Wv v + bv)^T                        (frame-major, bf16)
  per block j:  e = qc_j^T kc_win / 16      (64 x 128, fp32 psum)
                n = exp(e) * (mask + 1e-6); D = rowsum(n)  (fused DVE)
                m = n / D  (bf16) ; attT = transpose(m)    (PE)
                av = vt_win^T attT                          (psum)
  rav = relu(av) (bf16);  out = Wo rav + bo  (fp32)

Self-contained: hardcodes all shapes; imports only concourse + numpy.
"""

import sys
from contextlib import ExitStack

import numpy as np

for _p in ("/opt/trn_rl_repo", "/root/.axon_site/_ro/trn_rl_repo"):
    if _p not in sys.path:
        sys.path.append(_p)

import ml_dtypes  # noqa: E402

import concourse.bass as bass  # noqa: E402
from concourse import bacc  # noqa: E402
import concourse.mybir as mybir  # noqa: E402
import concourse.tile as tile  # noqa: E402

BF16 = ml_dtypes.bfloat16
F32 = mybir.dt.float32
BF = mybir.dt.bfloat16
Alu = mybir.AluOpType
Act = mybir.ActivationFunctionType

C = 512          # channels
CP = 256         # reduced channels
L = 32768        # full sequence
NCORES = 8
LLOC = L // NCORES          # 4096 frames per core
LKV = LLOC + 64             # 4160 k/v frames per core (32 halo each side)
BL = 64                     # block size
NCHUNK = 8                  # chunks per core
CH = 512                    # frames per chunk
NPAIR_C = 4                 # block-pairs per chunk

_NC_CACHE = {}


def _build_nc(reps=1, loop_k=None):
    nc = bacc.Bacc()
    d = {}
    inp = lambda name, shape, dt: nc.declare_dram_parameter(name, list(shape), dt, isOutput=False)
    d["qs"] = inp("qs", (C, LLOC), BF)
    d["ks"] = inp("ks", (C, LKV), BF)
    d["vs"] = inp("vs", (C, LKV), BF)
    d["wqt"] = inp("wqt", (C, CP), BF)
    d["wkt"] = inp("wkt", (C, CP), BF)
    d["wvt"] = inp("wvt", (C, CP), BF)
    d["wot"] = inp("wot", (CP, C), BF)
    d["bq2"] = inp("bq2", (CP, 1), F32)
    d["bk2"] = inp("bk2", (CP, 1), F32)
    d["bo4"] = inp("bo4", (C, 1), F32)
    d["bvb"] = inp("bvb", (128, CP), BF)
    d["w0"] = inp("w0", (128, 128), BF)
    d["wedge"] = inp("wedge", (2, 128, 128), BF)
    d["ident"] = inp("ident", (128, 128), BF)
    d["os"] = nc.declare_dram_parameter("os", [C, LLOC], F32, isOutput=True)

    with tile.TileContext(nc) as tc:
        with ExitStack() as ctx:
            _emit(ctx, tc, nc, d, reps, loop_k)
    nc.finalize()
    return nc


def _emit(ctx, tc, nc, d, reps, loop_k=None):
    # DRAM tiled views
    qs_v = d["qs"][:].rearrange("(t p) l -> p t l", p=128)   # [128, 4, 4096]
    ks_v = d["ks"][:].rearrange("(t p) l -> p t l", p=128)   # [128, 4, 4160]
    vs_v = d["vs"][:].rearrange("(t p) l -> p t l", p=128)
    os_v = d["os"][:].rearrange("(m p) l -> p m l", p=128)   # [128, 4, 4096]

    const = ctx.enter_context(tc.tile_pool(name="const", bufs=1))
    p_qin = ctx.enter_context(tc.tile_pool(name="qin", bufs=3))
    p_kin = ctx.enter_context(tc.tile_pool(name="kin", bufs=3))
    p_vin = ctx.enter_context(tc.tile_pool(name="vin", bufs=3))
    p_qc = ctx.enter_context(tc.tile_pool(name="qc", bufs=3))
    p_kc = ctx.enter_context(tc.tile_pool(name="kc", bufs=3))
    p_vt = ctx.enter_context(tc.tile_pool(name="vt", bufs=10))
    p_vts = ctx.enter_context(tc.tile_pool(name="vts", bufs=6))
    p_exp = ctx.enter_context(tc.tile_pool(name="exp", bufs=4))
    p_n = ctx.enter_context(tc.tile_pool(name="nn", bufs=4))
    p_dr = ctx.enter_context(tc.tile_pool(name="dr", bufs=6))
    p_m = ctx.enter_context(tc.tile_pool(name="mm", bufs=4))
    p_att = ctx.enter_context(tc.tile_pool(name="attT", bufs=4))
    p_rav = ctx.enter_context(tc.tile_pool(name="rav", bufs=3))
    p_out = ctx.enter_context(tc.tile_pool(name="outs", bufs=3))

    ps_qk = ctx.enter_context(tc.tile_pool(name="ps_qk", bufs=2, space="PSUM"))
    ps_v = ctx.enter_context(tc.tile_pool(name="ps_v", bufs=1, space="PSUM"))
    ps_en = ctx.enter_context(tc.tile_pool(name="ps_en", bufs=2, space="PSUM"))
    ps_mt = ctx.enter_context(tc.tile_pool(name="ps_mt", bufs=1, space="PSUM"))
    ps_av = ctx.enter_context(tc.tile_pool(name="ps_av", bufs=1, space="PSUM"))
    ps_fin = ctx.enter_context(tc.tile_pool(name="ps_fin", bufs=1, space="PSUM"))

    # ---- constants ----
    wqt_sb = const.tile([128, 4, CP], BF, tag="wqt")
    nc.sync.dma_start(out=wqt_sb[:], in_=d["wqt"][:].rearrange("(t p) c -> p t c", p=128))
    wkt_sb = const.tile([128, 4, CP], BF, tag="wkt")
    nc.sync.dma_start(out=wkt_sb[:], in_=d["wkt"][:].rearrange("(t p) c -> p t c", p=128))
    wvt_sb = const.tile([128, 4, CP], BF, tag="wvt")
    nc.sync.dma_start(out=wvt_sb[:], in_=d["wvt"][:].rearrange("(t p) c -> p t c", p=128))
    wot_sb = const.tile([128, 2, C], BF, tag="wot")
    nc.sync.dma_start(out=wot_sb[:], in_=d["wot"][:].rearrange("(t p) c -> p t c", p=128))
    bq_sb = const.tile([128, 2], F32, tag="bq")
    nc.sync.dma_start(out=bq_sb[:], in_=d["bq2"][:].rearrange("(t p) o -> p (t o)", p=128))
    bk_sb = const.tile([128, 2], F32, tag="bk")
    nc.sync.dma_start(out=bk_sb[:], in_=d["bk2"][:].rearrange("(t p) o -> p (t o)", p=128))
    bo_sb = const.tile([128, 4], F32, tag="bo")
    nc.sync.dma_start(out=bo_sb[:], in_=d["bo4"][:].rearrange("(t p) o -> p (t o)", p=128))
    bvr_sb = const.tile([1, CP], BF, tag="bvr")
    nc.sync.dma_start(out=bvr_sb[:], in_=d["bvr"][:])
    w0_sb = const.tile([128, 128], F32, tag="w0")
    nc.sync.dma_start(out=w0_sb[:], in_=d["w0"][:])
    wedge_sb = const.tile([128, 2, 128], F32, tag="wedge")
    nc.sync.dma_start(out=wedge_sb[:], in_=d["wedge"][:].rearrange("e p k -> p e k"))
    ident_sb = const.tile([128, 128], BF, tag="ident")
    nc.sync.dma_start(out=ident_sb[:], in_=d["ident"][:])
    ones_sb = const.tile([1, 128], BF, tag="ones")
    nc.vector.memset(ones_sb[:], 1.0)

    args = (tc, nc, d, qs_v, ks_v, vs_v, os_v,
            wqt_sb, wkt_sb, wvt_sb, wot_sb, bq_sb, bk_sb, bo_sb,
            bvb_sb, w0_sb, wedge_sb, ident_sb,
            p_qin, p_kin, p_vin, p_qc, p_kc, p_vt, p_vts,
            p_exp, p_n, p_dr, p_m, p_att, p_rav, p_out,
            ps_qk, ps_v, ps_en, ps_mt, ps_av, ps_fin)
    if loop_k is not None and loop_k > 1:
        with tc.For_i(0, loop_k, 1):
            _emit_body(*args)
    else:
        for _rep in range(reps):
            _emit_body(*args)


def _emit_body(tc, nc, d, qs_v, ks_v, vs_v, os_v,
               wqt_sb, wkt_sb, wvt_sb, wot_sb, bq_sb, bk_sb, bo_sb,
               bvb_sb, w0_sb, wedge_sb, ident_sb,
               p_qin, p_kin, p_vin, p_qc, p_kc, p_vt, p_vts,
               p_exp, p_n, p_dr, p_m, p_att, p_rav, p_out,
               ps_qk, ps_v, ps_en, ps_mt, ps_av, ps_fin):
    kc_tiles = {}     # chunk -> sbuf [128, 2, 576] bf16
    vt_tiles = {}     # 128-frame tile index -> sbuf [128, 256] bf16

    def load_kv(cc):
        """DMA k/v input chunk cc (512 frames, or 64-frame tail for cc==8)."""
        w = CH if cc < NCHUNK else 64
        kin = p_kin.tile([128, 4, CH], BF, tag="kin")
        nc.sync.dma_start(out=kin[:, :, :w], in_=ks_v[:, :, cc * CH:cc * CH + w])
        vin = p_vin.tile([128, 4, CH], BF, tag="vin")
        nc.sync.dma_start(out=vin[:, :, :w], in_=vs_v[:, :, cc * CH:cc * CH + w])
        return kin, vin

    def conv_k(cc, kin):
        """K-conv for chunk cc; fills kc_tiles[cc][:, :, 0:512] and patches
        kc_tiles[cc-1][:, :, 512:576]."""
        w = CH if cc < NCHUNK else 64
        if cc < NCHUNK:
            kc_tiles[cc] = p_kc.tile([128, 2, CH + 64], BF, tag="kc")
        for cpt in range(2):
            ps = ps_qk.tile([128, CH], F32, tag="psqk")
            for ct in range(4):
                nc.tensor.matmul(ps[:, :w],
                                 wkt_sb[:, ct, cpt * 128:(cpt + 1) * 128],
                                 kin[:, ct, :w],
                                 start=(ct == 0), stop=(ct == 3))
            if cc < NCHUNK:
                nc.scalar.activation(kc_tiles[cc][:, cpt, 0:CH], ps[:],
                                     Act.Identity, bias=bk_sb[:, cpt:cpt + 1])
                if cc > 0:
                    # patch previous chunk's 64-col tail
                    nc.vector.tensor_copy(kc_tiles[cc - 1][:, cpt, CH:CH + 64],
                                          kc_tiles[cc][:, cpt, 0:64])
            else:
                # 64-frame tail chunk: write directly into chunk 7's tail
                nc.scalar.activation(kc_tiles[cc - 1][:, cpt, CH:CH + 64], ps[:, 0:64],
                                     Act.Identity, bias=bk_sb[:, cpt:cpt + 1])

    def conv_v(cc, vin):
        """V-conv (transposed output) for chunk cc; fills vt_tiles[4*cc + t]."""
        nlt = 4 if cc < NCHUNK else 1
        w = 128 if cc < NCHUNK else 64
        for t in range(nlt):
            ps = ps_v.tile([128, CP], F32, tag="psv")
            for ct in range(4):
                nc.tensor.matmul(ps[:w, :],
                                 vin[:, ct, t * 128:t * 128 + w],
                                 wvt_sb[:, ct, :],
                                 start=(ct == 0), stop=False)
            nc.tensor.matmul(ps[:w, :], ones_sb[:, :w], bvr_sb[:],
                             start=False, stop=True)
            vt = p_vt.tile([128, CP], BF, tag="vt")
            nc.vector.tensor_copy(vt[:w, :], ps[:w, :])
            vt_tiles[4 * cc + t] = vt

    # ---- prologue: chunk 0 k/v ----
    kin, vin = load_kv(0)
    conv_k(0, kin)
    conv_v(0, vin)

    for c in range(NCHUNK):
        # 1) k/v conv one chunk ahead (incl. 64-frame tail at c == 7)
        kin, vin = load_kv(c + 1)
        conv_k(c + 1, kin)
        conv_v(c + 1, vin)

        # 2) shifted v tiles for the odd blocks of chunk c
        vts_tiles = {}
        for p in range(NPAIR_C):
            P = 4 * c + p
            vts = p_vts.tile([128, CP], BF, tag="vts")
            nc.gpsimd.dma_start(out=vts[0:64, :], in_=vt_tiles[P][64:128, :])
            nc.gpsimd.dma_start(out=vts[64:128, :], in_=vt_tiles[P + 1][0:64, :])
            vts_tiles[P] = vts

        # 3) q conv for chunk c
        qin = p_qin.tile([128, 4, CH], BF, tag="qin")
        nc.gpsimd.dma_start(out=qin[:], in_=qs_v[:, :, c * CH:(c + 1) * CH])
        qc = p_qc.tile([128, 2, CH], BF, tag="qc")
        for cpt in range(2):
            ps = ps_qk.tile([128, CH], F32, tag="psqk")
            for ct in range(4):
                nc.tensor.matmul(ps[:],
                                 wqt_sb[:, ct, cpt * 128:(cpt + 1) * 128],
                                 qin[:, ct, :],
                                 start=(ct == 0), stop=(ct == 3))
            nc.scalar.activation(qc[:, cpt, :], ps[:], Act.Identity,
                                 bias=bq_sb[:, cpt:cpt + 1])

        # 4) attention per block-pair
        kc = kc_tiles[c]
        rav = p_rav.tile([128, 2, CH], BF, tag="rav")
        for p in range(NPAIR_C):
            P = 4 * c + p
            # energy for the two blocks, stacked on partitions
            ps_e = ps_en.tile([128, 128], F32, tag="psen")
            for half in range(2):
                jj = 2 * p + half
                for cpt in range(2):
                    nc.tensor.matmul(ps_e[half * 64:(half + 1) * 64, :],
                                     qc[:, cpt, jj * 64:jj * 64 + 64],
                                     kc[:, cpt, jj * 64:jj * 64 + 128],
                                     start=(cpt == 0), stop=(cpt == 1))
            # softmax (unnormalized weights n, row-sums D via fused op)
            if P == 0:
                Wsel = wedge_sb[:, 0, :]
            elif P == NCHUNK * NPAIR_C - 1:
                Wsel = wedge_sb[:, 1, :]
            else:
                Wsel = w0_sb[:]
            expt = p_exp.tile([128, 128], F32, tag="exp")
            nc.scalar.activation(expt[:], ps_e[:], Act.Exp, scale=1.0 / 16.0)
            nt = p_n.tile([128, 128], F32, tag="nn")
            Dt = p_dr.tile([128, 1], F32, tag="D")
            nc.vector.scalar_tensor_tensor(nt[:], expt[:], 0.0, Wsel,
                                           op0=Alu.bypass, op1=Alu.mult,
                                           accum_out=Dt[:])
            rt = p_dr.tile([128, 1], F32, tag="r")
            nc.vector.reciprocal(rt[:], Dt[:])
            mt = p_m.tile([128, 128], BF, tag="mm")
            nc.vector.tensor_scalar_mul(mt[:], nt[:], rt[:])
            # transpose att (PE) then copy to SBUF
            ps_m = ps_mt.tile([128, 128], BF, tag="psmt")
            nc.tensor.transpose(ps_m[:], mt[:], ident_sb[:])
            attT = p_att.tile([128, 128], BF, tag="attT")
            nc.vector.tensor_copy(attT[:], ps_m[:])
            # att @ V  -> [cp, 128] for the pair
            ps_a = ps_av.tile([128, 2, 128], F32, tag="psav")
            for cpt in range(2):
                nc.tensor.matmul(ps_a[:, cpt, 0:64],
                                 vt_tiles[P][:, cpt * 128:(cpt + 1) * 128],
                                 attT[:, 0:64], start=True, stop=True)
                nc.tensor.matmul(ps_a[:, cpt, 64:128],
                                 vts_tiles[P][:, cpt * 128:(cpt + 1) * 128],
                                 attT[:, 64:128], start=True, stop=True)
            # relu -> rav (alternate engines for balance)
            for cpt in range(2):
                dst = rav[:, cpt, p * 128:(p + 1) * 128]
                nc.vector.tensor_scalar_max(dst, ps_a[:, cpt, :], 0.0)

        # 5) output conv
        outs = p_out.tile([128, 4, CH], F32, tag="outs")
        for m in range(4):
            ps = ps_fin.tile([128, CH], F32, tag="psfin")
            for kt in range(2):
                nc.tensor.matmul(ps[:],
                                 wot_sb[:, kt, m * 128:(m + 1) * 128],
                                 rav[:, kt, :],
                                 start=(kt == 0), stop=(kt == 1))
            if m < 2:
                nc.scalar.activation(outs[:, m, :], ps[:], Act.Identity,
                                     bias=bo_sb[:, m:m + 1])
            else:
                nc.vector.tensor_scalar_add(outs[:, m, :], ps[:], bo_sb[:, m:m + 1])
        nc.sync.dma_start(out=os_v[:, 0:2, c * CH:(c + 1) * CH], in_=outs[:, 0:2, :])
        nc.scalar.dma_start(out=os_v[:, 2:4, c * CH:(c + 1) * CH], in_=outs[:, 2:4, :])


def _window_mask():
    i = np.arange(BL)[:, None]
    j = np.arange(2 * BL)[None, :]
    return ((j >= i) & (j < i + BL)).astype(np.float32)


def _host_shard(q, k, v, mask, Wq, bq, Wk, bk, Wv, bv, Wo, bo):
    half = BL // 2
    q0 = np.asarray(q[0], np.float32)
    kp = np.pad(np.asarray(k[0], np.float32), ((0, 0), (half, half)))
    vp = np.pad(np.asarray(v[0], np.float32), ((0, 0), (half, half)))
    pmp = np.pad(np.asarray(mask[0, 0], np.float32), (half, half))
    wm = _window_mask()
    w0 = (np.concatenate([wm, wm], axis=0) + 1e-6).astype(np.float32)
    shared = {
        "wqt": np.ascontiguousarray(Wq.T).astype(BF16),
        "wkt": np.ascontiguousarray(Wk.T).astype(BF16),
        "wvt": np.ascontiguousarray(Wv.T).astype(BF16),
        "wot": np.ascontiguousarray(Wo.T).astype(BF16),
        "bq2": np.asarray(bq, np.float32).reshape(CP, 1),
        "bk2": np.asarray(bk, np.float32).reshape(CP, 1),
        "bo4": np.asarray(bo, np.float32).reshape(C, 1),
        "bvb": np.tile(np.asarray(bv, np.float32).reshape(1, CP), (128, 1)).astype(BF16),
        "ident": np.eye(128, dtype=BF16),
    }
    in_maps = []
    for r in range(NCORES):
        s = r * LLOC
        pml = pmp[s:s + LKV]

        def wpair(P):
            r0 = wm * pml[128 * P:128 * P + 128][None, :]
            r1 = wm * pml[128 * P + 64:128 * P + 192][None, :]
            return np.concatenate([r0, r1], axis=0) + 1e-6

        # wmask3[e][p, u, k]: e=0 interior (w0|w0), e=1 first batch
        # (wpair0|w0), e=2 last batch (w0|wpair31)
        wmask3 = np.stack([
            np.stack([w0, w0], axis=1),
            np.stack([wpair(0), w0], axis=1),
            np.stack([w0, wpair(31)], axis=1),
        ]).astype(BF16)
        in_maps.append({
            "qs": q0[:, s:s + LLOC].astype(BF16),
            "ks": kp[:, s:s + LKV].astype(BF16),
            "vs": vp[:, s:s + LKV].astype(BF16),
            "wmask3": wmask3,
            **shared,
        })
    return in_maps


def kernel(q, k, v, mask, Wq, bq, Wk, bk, Wv, bv, Wo, bo):
    from concourse.bass_utils import run_bass_kernel_spmd

    if "nc" not in _NC_CACHE:
        _NC_CACHE["nc"] = _build_nc()
    nc = _NC_CACHE["nc"]
    in_maps = _host_shard(q, k, v, mask, Wq, bq, Wk, bk, Wv, bv, Wo, bo)
    res = run_bass_kernel_spmd(nc, in_maps, list(range(NCORES)))
    out = np.concatenate([res.results[r]["os"] for r in range(NCORES)], axis=1)
    out = out[None].astype(np.float32)
    return out * np.asarray(mask, np.float32)[:, 0:1, :]
